# revision 10
# baseline (speedup 1.0000x reference)
"""Single-head full attention (B=4, S=4096, D=512) on 8 TRN2 NeuronCores.

Sharding: core c handles batch b = c//2, query half h = c%2 (2048 queries).

Algebraic folds:
  scores = (x_q Wq^T)(x Wk^T)^T / sqrt(D) = x_q @ M @ x^T,  M = Wq^T Wk / sqrt(D)
    -> K is never materialized; x^T (resident, fp16) is the stationary operand
       and T = x_q @ M replaces Q.
  y = P V = P (x Wv^T) = (P x) Wv^T
    -> V is never materialized either (the baseline computed the full V
       projection redundantly on both cores of a batch). Instead z^T = (P x)^T
       accumulates directly from pexp tiles (already [j, q]-oriented), then
       y^T = Wv z^T needs no transposes at all. Output leaves the device as
       y^T (unnormalized) + per-qt partial sums of exp; the host divides by
       the softmax denominator and transposes.

Device layouts (per core, fp16 operands, fp32 accumulate):
  xt_sb [128, 4, 4096]: x^T, partition p + tile t -> d' = t*128+p (query-half
        columns DMA'd first so the T projection starts immediately)
  xn_sb [128, 32, 512]: x natural, partition p + block jb -> j = jb*128+p
  tt_sb [128, 4, 2048]: T^T = (x_q @ M)^T
Scores are computed transposed (S^T[j, q]); exp(S^T) tiles feed the z^T
matmul as the moving operand. Softmax denominators are elementwise-
accumulated over key blocks on the vector engine (acc[p, q] = sum_jb pexp)
and reduced/normalized on the host, so the PE spends zero cycles on them.
No max-subtraction: scores are O(1) and softmax is shift-invariant.

PSUM (8 banks): 2 (scores/proj rotating) + 4 (z^T accumulators) + 2 (y^T).
"""
import math
import numpy as np

B, S, D = 4, 4096, 512
P = 128
SQ = S // 2          # queries per core
NCORES = 8
QTILE = 512          # query columns per score/zt pass

last_results = None  # BassKernelResults of the most recent run (for test.py)

_nc_cache = {}


def _build_nc(has_bias, has_mask, reps=1):
    import concourse.bacc as bacc
    import concourse.tile as tile
    from concourse import mybir
    from contextlib import ExitStack

    f32 = mybir.dt.float32
    f16 = mybir.dt.float16
    Exp = mybir.ActivationFunctionType.Exp

    nc = bacc.Bacc("TRN2", target_bir_lowering=False, debug=False)
    xT = nc.declare_dram_parameter("xT", [D, S], f16, False)
    xN = nc.declare_dram_parameter("xN", [S, D], f16, False)
    mT = nc.declare_dram_parameter("mT", [D, D], f16, False)
    wvT = nc.declare_dram_parameter("wvT", [D, D], f16, False)
    if has_bias:
        wtl = nc.declare_dram_parameter("wtl", [P, D // P], f16, False)
    if has_mask:
        maskf = nc.declare_dram_parameter("maskf", [P, S // P], f32, False)
    ytT = nc.declare_dram_parameter("ytT", [D, SQ], f32, True)
    accO = nc.declare_dram_parameter("accO", [P, SQ], f32, True)

    ET = D // P          # 4 d'-tiles
    NJB = S // P         # 32 key blocks
    NQT = SQ // QTILE    # 4 query tiles
    NDC = D // P         # 4 d-chunks of z^T / e-chunks of y^T

    with tile.TileContext(nc) as tc, ExitStack() as ctx:
        wpool = ctx.enter_context(tc.tile_pool(name="wpool", bufs=1))
        big = ctx.enter_context(tc.tile_pool(name="big", bufs=1))
        expp = ctx.enter_context(tc.tile_pool(name="expp", bufs=6))
        accp = ctx.enter_context(tc.tile_pool(name="accp", bufs=2))
        ztp = ctx.enter_context(tc.tile_pool(name="ztp", bufs=2))
        outp = ctx.enter_context(tc.tile_pool(name="outp", bufs=4))
        smallp = ctx.enter_context(tc.tile_pool(name="smallp", bufs=3))
        psum_s = ctx.enter_context(tc.tile_pool(name="psum_s", bufs=2, space="PSUM"))
        psum_zt = ctx.enter_context(tc.tile_pool(name="psum_zt", bufs=1, space="PSUM"))
        psum_ot = ctx.enter_context(tc.tile_pool(name="psum_ot", bufs=2, space="PSUM"))

        m_sb = wpool.tile([P, ET, D], f16)
        wv_sb = wpool.tile([P, ET, D], f16)
        nc.sync.dma_start(out=m_sb, in_=mT[:, :].rearrange("(t p) e -> p t e", p=P))
        if has_bias:
            wtl_sb = wpool.tile([P, D // P], f16)
            nc.sync.dma_start(out=wtl_sb, in_=wtl[:, :])
        if has_mask:
            mask_sb = wpool.tile([P, S // P], f32)
            nc.sync.dma_start(out=mask_sb, in_=maskf[:, :])

        xt_sb = big.tile([P, ET, S], f16)
        xn_sb = big.tile([P, NJB, D], f16)
        tt_sb = big.tile([P, ET, SQ], f16)

        xT_r = xT[:, :].rearrange("(t p) s -> p t s", p=P)
        xN_r = xN[:, :].rearrange("(jb p) d -> p jb d", p=P)

        def body(rep):
            # resident x^T load. The host rotates keys so this core's query
            # half is always columns [0, SQ) — the same program serves all 8
            # cores (softmax is invariant to key order as long as xT and xN
            # share the rotation). Chunked so consumers unlock early; x
            # natural interleaved (needed from the first z^T matmul).
            for i in range(S // QTILE):
                c0 = i * QTILE
                nc.sync.dma_start(
                    out=xt_sb[:, :, c0:c0 + QTILE],
                    in_=xT_r[:, :, c0:c0 + QTILE])
                jb0 = i * (NJB // 8)
                nc.sync.dma_start(
                    out=xn_sb[:, jb0:jb0 + NJB // 8, :],
                    in_=xN_r[:, jb0:jb0 + NJB // 8, :])
                if i == 1 and rep == 0:
                    # Wv^T isn't needed until the first y^T matmul (~60us in);
                    # keep it off the startup critical path.
                    nc.sync.dma_start(
                        out=wv_sb, in_=wvT[:, :].rearrange("(t p) e -> p t e", p=P))

            # T^T projection: M-stationary, x_q^T-moving
            for c in range(SQ // QTILE):
                for me in range(ET):
                    pq = psum_s.tile([P, QTILE], f32, tag="mm512",
                                     name=f"pq_{rep}_{c}_{me}")
                    for t in range(ET):
                        nc.tensor.matmul(
                            pq,
                            lhsT=m_sb[:, t, me * P:(me + 1) * P],
                            rhs=xt_sb[:, t, c * QTILE:(c + 1) * QTILE],
                            start=(t == 0), stop=(t == ET - 1))
                    nc.scalar.copy(out=tt_sb[:, me, c * QTILE:(c + 1) * QTILE], in_=pq)

            # per-key bias multiplier exp(beta[j]) (only when biases present)
            if has_bias:
                bmul_sb = smallp.tile([P, NJB], f32, tag="bmul", name=f"bm_{rep}")
                for jb in range(NJB):
                    pb = psum_s.tile([P, QTILE], f32, tag="mm512",
                                     name=f"pb_{rep}_{jb}")
                    for t in range(ET):
                        nc.tensor.matmul(
                            pb[:, 0:2],
                            lhsT=xt_sb[:, t, jb * P:(jb + 1) * P],
                            rhs=wtl_sb[:, t:t + 1].to_broadcast([P, 2]),
                            start=(t == 0), stop=(t == ET - 1))
                    nc.scalar.activation(out=bmul_sb[:, jb:jb + 1], in_=pb[:, 0:1],
                                         func=Exp, scale=1.0)

            # attention: scores^T -> exp -> z^T = (P x)^T, denominator on DVE
            for qt in range(NQT):
                zt = [psum_zt.tile([P, QTILE], f32, tag=f"zt{dc}",
                                   name=f"zt_{rep}_{qt}_{dc}")
                      for dc in range(NDC)]
                acc = accp.tile([P, QTILE], f32, tag="acc", name=f"acc_{rep}_{qt}")
                for jb in range(NJB):
                    ps_t = psum_s.tile([P, QTILE], f32, tag="mm512",
                                       name=f"ps_{rep}_{qt}_{jb}")
                    for t in range(ET):
                        nc.tensor.matmul(
                            ps_t,
                            lhsT=xt_sb[:, t, jb * P:(jb + 1) * P],
                            rhs=tt_sb[:, t, qt * QTILE:(qt + 1) * QTILE],
                            start=(t == 0), stop=(t == ET - 1))
                    pexp = expp.tile([P, QTILE], f16, tag="pexp",
                                     name=f"pe_{rep}_{qt}_{jb}")
                    nc.scalar.activation(out=pexp, in_=ps_t, func=Exp, scale=1.0)
                    if has_bias:
                        nc.vector.tensor_scalar_mul(pexp, pexp, bmul_sb[:, jb:jb + 1])
                    if has_mask:
                        nc.vector.tensor_scalar_mul(pexp, pexp, mask_sb[:, jb:jb + 1])
                    if jb == 0:
                        nc.vector.tensor_copy(out=acc, in_=pexp)
                    else:
                        nc.vector.tensor_add(out=acc, in0=acc, in1=pexp)
                    for dc in range(NDC):
                        nc.tensor.matmul(
                            zt[dc],
                            lhsT=xn_sb[:, jb, dc * P:(dc + 1) * P],
                            rhs=pexp,
                            start=(jb == 0), stop=(jb == NJB - 1))
                nc.sync.dma_start(out=accO[:, qt * QTILE:(qt + 1) * QTILE], in_=acc)

                ztsb = ztp.tile([P, NDC, QTILE], f16, tag="ztsb", name=f"zs_{rep}_{qt}")
                for dc in range(NDC):
                    nc.vector.tensor_copy(out=ztsb[:, dc, :], in_=zt[dc])
                # y^T = Wv @ z^T
                for ec in range(NDC):
                    ot = psum_ot.tile([P, QTILE], f32, tag="ot",
                                      name=f"ot_{rep}_{qt}_{ec}")
                    for dc in range(NDC):
                        nc.tensor.matmul(
                            ot,
                            lhsT=wv_sb[:, dc, ec * P:(ec + 1) * P],
                            rhs=ztsb[:, dc, :],
                            start=(dc == 0), stop=(dc == NDC - 1))
                    osb = outp.tile([P, QTILE], f32, tag="osb",
                                    name=f"o_{rep}_{qt}_{ec}")
                    nc.vector.tensor_copy(out=osb, in_=ot)
                    nc.sync.dma_start(
                        out=ytT[ec * P:(ec + 1) * P, qt * QTILE:(qt + 1) * QTILE],
                        in_=osb)

        if reps == 1:
            body(0)
        else:
            # bench-only loop; hint the big-body engines so the back-edge
            # branch prefetches its IRAM block instead of stalling ~4us
            with tc.For_i(0, reps, 1,
                          hint_engines=(mybir.EngineType.PE,
                                        mybir.EngineType.Activation,
                                        mybir.EngineType.DVE,
                                        mybir.EngineType.SP)):
                body(0)
    nc.compile()
    return nc


def _prepare(x, mask, Wq, bq, Wk, bk, Wv, bv):
    """Build (or fetch cached) device program + per-core input maps."""
    x = np.asarray(x, dtype=np.float32)
    mask = np.asarray(mask)
    Wq = np.asarray(Wq, dtype=np.float32)
    Wk = np.asarray(Wk, dtype=np.float32)
    Wv = np.asarray(Wv, dtype=np.float32)
    bq = np.asarray(bq, dtype=np.float32)
    bk = np.asarray(bk, dtype=np.float32)
    bv = np.asarray(bv, dtype=np.float32)
    has_bias = bool(np.any(bq) or np.any(bk) or np.any(bv))
    has_mask = bool(np.any(mask))

    key = (has_bias, has_mask)
    if key not in _nc_cache:
        _nc_cache[key] = _build_nc(has_bias, has_mask)
    nc = _nc_cache[key]

    inv_sqrt_d = 1.0 / math.sqrt(D)
    M = (Wq.T.astype(np.float64) @ Wk.astype(np.float64)) * inv_sqrt_d
    mT_h = np.ascontiguousarray(M.astype(np.float32).astype(np.float16))
    wvT_h = np.ascontiguousarray(Wv.T.astype(np.float16))

    in_maps = []
    for c in range(NCORES):
        b, h = divmod(c, 2)
        # rotate keys so this core's query half sits in rows [0, SQ) — the
        # single SPMD program always reads queries from there; softmax over
        # keys is order-invariant as long as xT and xN share the rotation.
        xr = np.roll(x[b], -h * SQ, axis=0).astype(np.float16)
        m = {
            "xT": np.ascontiguousarray(xr.T),
            "xN": np.ascontiguousarray(xr),
            "mT": mT_h, "wvT": wvT_h,
        }
        if has_bias:
            # per-key additive beta[j] = (bq Wk/sqrt(D)).x[j]; the bq.bk
            # constant shifts all keys equally and cancels in softmax.
            wt = (bq @ Wk) * inv_sqrt_d              # [D]
            m["wtl"] = np.ascontiguousarray(
                wt.reshape(D // P, P).T.astype(np.float16))
        if has_mask:
            keep = 1.0 - np.roll(mask[b], -h * SQ).astype(np.float32)
            m["maskf"] = np.ascontiguousarray(keep.reshape(S // P, P).T)
        in_maps.append(m)
    return nc, in_maps, bv if has_bias else None


def _gather(res, bv):
    out = np.empty((B, S, D), dtype=np.float32)
    for c in range(NCORES):
        b, h = divmod(c, 2)
        ytT = res.results[c]["ytT"]                  # [D, SQ] unnormalized y^T
        den = res.results[c]["accO"].sum(axis=0)     # [SQ] softmax denominators
        y = (ytT / den[None, :]).T
        if bv is not None:
            y = y + bv[None, :]
        out[b, h * SQ:(h + 1) * SQ, :] = y
    return out


def kernel(x, mask, Wq, bq, Wk, bk, Wv, bv):
    global last_results
    from concourse.bass_utils import run_bass_kernel_spmd

    nc, in_maps, bv_h = _prepare(x, mask, Wq, bq, Wk, bk, Wv, bv)
    res = run_bass_kernel_spmd(nc, in_maps, core_ids=list(range(NCORES)))
    last_results = res
    return _gather(res, bv_h)
